# revision 21
# baseline (speedup 1.0000x reference)
"""Trainium2 Bass kernel for nn_LinearMixtureOfMixers.

Strategy: data-parallel over batch B=8 across the 8 NeuronCores (one batch
element per core). The router (mean-pool -> (8,8) logits -> softmax -> top-2)
and the scalar aux loss are computed on host; routing drives a host-side
gather of each core's two (H,N,N) expert tables, which are shiped
pre-transposed so the device kernel needs no on-chip transposes anywhere:

  per core (batch b), with N=512 tokens, D=1024, H=8 heads, HD=128:
    xp[n,d]   = sum_c x[n,c] in_w[d,c]          PE  (lhsT = x.T blocks, rhs = in_w.T)
                (in_b is dropped: a per-d shift is removed exactly by the
                 token-axis layernorm that follows)
    stats     = ones-matmul partition reduction  PE  -> broadcast mu/rsqrt tiles
    xn[n,hd]  = (xp - mu) * rsqrt(var+eps)       DVE
    expw[n,m] = exp(wT[k,h])                     ACT (in-place, no max needed:
                                                      |w| <= 1/sqrt(N))
    U[d,m]    = sum_n xn[n,(h,d)] expw[n,m]      PE  (accumulate 4 n-chunks)
    rs[*,m]   = sum_n expw[n,m] / topw_k         PE  (stationary = 1/topw_k)
    outh[d,m] = U0*recip(rs0) + U1*recip(rs1) + bias_eff  DVE
    y[m,f]    = sum_hd outh[hd,m] outwT[hd,f] + out_b     PE (outh is lhsT
                 directly; out_b enters as a K=1 matmul)
"""

import os

import numpy as np

import concourse.bass as bass
import concourse.tile as tile
from concourse import mybir
from concourse.bass_utils import run_bass_kernel_spmd

F32 = mybir.dt.float32
P = 128
B, N, D = 8, 512, 1024
E, H, TOP_K = 8, 8, 2
HD = D // H  # 128
NCH = N // P  # 4 n-chunks
DCH = D // P  # 8 c-chunks
EPS = 1e-5


# ---------------------------------------------------------------------------
# Workaround: this container's walrus rejects >2 sem waits on the Tile tail
# Drain. Emit one Drain per waiting proc instead (semantically identical).
def _patch_tile_drain():
    from concourse.vector_clock import ScopedClock, VectorClock

    def _drain_and_barrier_split(self, tick_clock, wait_clock):
        nc = self.nc
        vc = tick_clock.global_clock
        n = len(vc)
        for proc in range(n):
            t = vc[proc]
            if t > 0:
                single = VectorClock([t if i == proc else 0 for i in range(n)])
                d = nc.sync.drain()
                wait_clock.add_sem_waits(d.ins, ScopedClock({None: single}))
        nc.all_engine_barrier()
        assert self.sems is not None
        popped = nc._tile_sem_poison_stack.pop()
        assert popped is self._sem_poison
        nc.clear_and_free_semaphores(list(self.sems.allocated().values()))
        nc.all_engine_barrier()

    tile.TileContext._drain_and_barrier = _drain_and_barrier_split


_patch_tile_drain()


MAX_WAITS = 1  # walrus in this container caps sync waits per instruction


def _split_excess_waits(nc):
    """Hoist >MAX_WAITS sem waits onto same-engine NoOps placed just before."""
    n_split = 0
    for fn in nc.m.functions:
        for bb in fn.blocks:
            out = []
            for inst in bb.instructions:
                si = inst.sync_info
                if si is not None and si.on_wait and len(si.on_wait) > MAX_WAITS:
                    waits = list(si.on_wait)
                    extras, keep = waits[:-MAX_WAITS], waits[-MAX_WAITS:]
                    for i in range(0, len(extras), MAX_WAITS):
                        nop = mybir.InstDrain(
                            name=f"{inst.name}_wsplit{n_split}",
                            sync_info=mybir.SyncInfo(
                                on_wait=extras[i : i + MAX_WAITS], on_update=[]
                            ),
                            engine=inst.engine,
                        )
                        out.append(nop)
                        n_split += 1
                    inst.sync_info = mybir.SyncInfo(
                        on_wait=keep, on_update=si.on_update
                    )
                out.append(inst)
            bb.instructions = out
    return n_split


def build_nc():
    nc = bass.Bass()
    xt_d = nc.dram_tensor("xt", [D, N], F32, kind="ExternalInput")
    inwt_d = nc.dram_tensor("inwt", [D, D], F32, kind="ExternalInput")
    outwt_d = nc.dram_tensor("outwt", [D, D], F32, kind="ExternalInput")
    outb_d = nc.dram_tensor("outb", [D], F32, kind="ExternalInput")
    wsel_d = nc.dram_tensor("wsel", [TOP_K, H, N, N], F32, kind="ExternalInput")
    bsel_d = nc.dram_tensor("bsel", [TOP_K, H, N], F32, kind="ExternalInput")
    # tw[k, 0, :] = topw_k (replicated), tw[k, 1, :] = 1/topw_k (replicated)
    tw_d = nc.dram_tensor("tw", [TOP_K, 2, N], F32, kind="ExternalInput")
    out_d = nc.dram_tensor("out", [N, D], F32, kind="ExternalOutput")

    with tile.TileContext(nc) as tc:
        from contextlib import ExitStack

        with ExitStack() as ctx:
            p_const = ctx.enter_context(tc.tile_pool(name="const", bufs=1))
            p_inw = ctx.enter_context(tc.tile_pool(name="inw", bufs=3))
            p_outw = ctx.enter_context(tc.tile_pool(name="outw", bufs=3))
            p_xt = ctx.enter_context(tc.tile_pool(name="xt", bufs=3))
            p_xp = ctx.enter_context(tc.tile_pool(name="xp", bufs=NCH))
            p_sq = ctx.enter_context(tc.tile_pool(name="sq", bufs=4))
            p_stat = ctx.enter_context(tc.tile_pool(name="stat", bufs=1))
            p_w = ctx.enter_context(tc.tile_pool(name="w", bufs=4))
            p_outh = ctx.enter_context(tc.tile_pool(name="outh", bufs=H))
            p_small = ctx.enter_context(tc.tile_pool(name="small", bufs=3))
            p_y = ctx.enter_context(tc.tile_pool(name="y", bufs=2))
            p_ps = ctx.enter_context(tc.tile_pool(name="ps", bufs=4, space="PSUM"))
            p_dram = ctx.enter_context(tc.tile_pool(name="dram", bufs=1, space="DRAM"))

            # ---- constants & tiny inputs -------------------------------
            ones_inv = p_const.tile([P, P], F32, tag="ones_inv")
            nc.any.memset(ones_inv[:], 1.0 / N)
            ones_row = p_const.tile([1, P], F32, tag="ones_row")
            nc.any.memset(ones_row[:], 1.0)
            eps_col = p_const.tile([P, 1], F32, tag="eps_col")
            nc.any.memset(eps_col[:], EPS)

            outb_sb = p_const.tile([1, D], F32, tag="outb_sb")
            nc.sync.dma_start(outb_sb[:], outb_d[:].unsqueeze(0))

            itw = []
            for k in range(TOP_K):
                t = p_const.tile([P, P], F32, tag=f"itw{k}")
                nc.sync.dma_start(t[:], tw_d[k : k + 1, 1, 0:P].to_broadcast((P, P)))
                itw.append(t)
            twp = []
            for k in range(TOP_K):
                t = p_const.tile([H, 1], F32, tag=f"twp{k}")
                nc.sync.dma_start(t[:], tw_d[k : k + 1, 0, 0:1].to_broadcast((H, 1)))
                twp.append(t)

            bsk = []
            for k in range(TOP_K):
                t = p_const.tile([H, N], F32, tag=f"bsk{k}")
                nc.sync.dma_start(t[:], bsel_d[k])
                bsk.append(t)
            # bias_eff[h,m] = topw0*b0[h,m] + topw1*b1[h,m]
            beff = p_const.tile([H, N], F32, tag="beff")
            tmpb = p_const.tile([H, N], F32, tag="tmpb")
            nc.vector.tensor_scalar_mul(tmpb[:], bsk[0][:], twp[0][:])
            nc.vector.scalar_tensor_tensor(
                beff[:], bsk[1][:], twp[1][:], tmpb[:],
                mybir.AluOpType.mult, mybir.AluOpType.add,
            )
            # bounce beff to DRAM: DMA partition-broadcast needs a DRAM source
            beff_dr = p_dram.tile([H, N], F32, tag="beff_dr")
            nc.sync.dma_start(beff_dr[:], beff[:])

            # ---- input projection: xp[n,d] (4 chunks of [128, 1024]) ---
            # c-outer loop so xt/inw stream through small double-buffers.
            ps_xp = [p_ps.tile([P, D], F32, tag="ps", name=f"ps_xp{i}") for i in range(NCH)]
            for c in range(DCH):
                xt_t = p_xt.tile([P, N], F32, tag="xt")
                nc.sync.dma_start(xt_t[:], xt_d[c * P : (c + 1) * P, :])
                inw_t = p_inw.tile([P, D], F32, tag="inw")
                nc.sync.dma_start(inw_t[:], inwt_d[c * P : (c + 1) * P, :])
                for nc_i in range(NCH):
                    for half in range(2):
                        nc.tensor.matmul(
                            ps_xp[nc_i][:, half * 512 : (half + 1) * 512],
                            xt_t[:, nc_i * P : (nc_i + 1) * P],
                            inw_t[:, half * 512 : (half + 1) * 512],
                            start=(c == 0),
                            stop=(c == DCH - 1),
                        )
            xp = []
            sq = []
            for nc_i in range(NCH):
                t = p_xp.tile([P, D], F32, tag="xp")
                nc.scalar.copy(t[:], ps_xp[nc_i][:])
                xp.append(t)
                s = p_sq.tile([P, D], F32, tag="sq")
                nc.scalar.square(s[:], t[:])
                sq.append(s)

            # ---- layernorm stats over tokens (partition axis) ----------
            ps_mu = p_ps.tile([P, D], F32, tag="ps")
            ps_m2 = p_ps.tile([P, D], F32, tag="ps")
            for nc_i in range(NCH):
                for half in range(2):
                    sl = slice(half * 512, (half + 1) * 512)
                    nc.tensor.matmul(
                        ps_mu[:, sl], ones_inv[:], xp[nc_i][:, sl],
                        start=(nc_i == 0), stop=(nc_i == NCH - 1),
                    )
                    nc.tensor.matmul(
                        ps_m2[:, sl], ones_inv[:], sq[nc_i][:, sl],
                        start=(nc_i == 0), stop=(nc_i == NCH - 1),
                    )

            muB = p_stat.tile([P, D], F32, tag="muB")
            nc.scalar.copy(muB[:], ps_mu[:])
            mu2 = p_stat.tile([P, D], F32, tag="mu2")
            nc.vector.tensor_mul(mu2[:], muB[:], muB[:])
            varB = p_stat.tile([P, D], F32, tag="varB")
            nc.vector.tensor_sub(varB[:], ps_m2[:], mu2[:])
            # rsqrt(v+eps) = exp(-0.5*ln(v+eps)); ACT Ln/Exp are ~2ULP tables
            lnv = p_stat.tile([P, D], F32, tag="lnv")
            nc.scalar.activation(
                lnv[:], varB[:], mybir.ActivationFunctionType.Ln, bias=eps_col[:]
            )
            rsB = p_stat.tile([P, D], F32, tag="rsB")
            nc.scalar.activation(
                rsB[:], lnv[:], mybir.ActivationFunctionType.Exp, scale=-0.5
            )

            # xn in place of xp: (xp - mu) * rs
            for nc_i in range(NCH):
                nc.vector.tensor_sub(xp[nc_i][:], xp[nc_i][:], muB[:])
                nc.vector.tensor_mul(xp[nc_i][:], xp[nc_i][:], rsB[:])
            xn = xp

            # ---- experts: exp, U-einsum, rowsums, combine --------------
            for h in range(H):
                wk = []
                for k in range(TOP_K):
                    wt = p_w.tile([P, NCH, N], F32, tag="w")
                    nc.sync.dma_start(
                        wt[:],
                        wsel_d[k, h].rearrange("(c p) m -> p c m", p=P),
                    )
                    nc.scalar.activation(
                        wt[:], wt[:], mybir.ActivationFunctionType.Exp
                    )
                    wk.append(wt)

                ps_u = p_ps.tile([P, D], F32, tag="ps")  # halves: U0 | U1
                ps_r = p_ps.tile([P, D], F32, tag="ps")  # halves: rs0 | rs1
                for nc_i in range(NCH):
                    for k in range(TOP_K):
                        nc.tensor.matmul(
                            ps_u[:, k * 512 : (k + 1) * 512],
                            xn[nc_i][:, h * HD : (h + 1) * HD],
                            wk[k][:, nc_i, :],
                            start=(nc_i == 0),
                            stop=(nc_i == NCH - 1),
                        )
                for nc_i in range(NCH):
                    for k in range(TOP_K):
                        nc.tensor.matmul(
                            ps_r[:, k * 512 : (k + 1) * 512],
                            itw[k][:],
                            wk[k][:, nc_i, :],
                            start=(nc_i == 0),
                            stop=(nc_i == NCH - 1),
                        )

                # recip_k = topw_k / rowsum_k  (ps_r is already rowsum/topw):
                # 1/x = exp(-ln x) on ACT (~2ULP tables)
                rec = []
                for k in range(TOP_K):
                    lnr = p_small.tile([P, N], F32, tag="rscratch")
                    nc.scalar.activation(
                        lnr[:],
                        ps_r[:, k * 512 : (k + 1) * 512],
                        mybir.ActivationFunctionType.Ln,
                    )
                    r = p_small.tile([P, N], F32, tag="rec")
                    nc.scalar.activation(
                        r[:], lnr[:], mybir.ActivationFunctionType.Exp, scale=-1.0
                    )
                    rec.append(r)

                biasB = p_small.tile([P, N], F32, tag="biasB")
                nc.sync.dma_start(biasB[:], beff_dr[h : h + 1, :].to_broadcast((P, N)))

                m0 = p_small.tile([P, N], F32, tag="m0")
                nc.vector.tensor_mul(m0[:], ps_u[:, 0:512], rec[0][:])
                m1 = p_small.tile([P, N], F32, tag="m1")
                nc.vector.tensor_mul(m1[:], ps_u[:, 512:1024], rec[1][:])
                oh = p_outh.tile([P, N], F32, tag="outh")
                nc.vector.tensor_add(oh[:], m0[:], m1[:])
                nc.vector.tensor_add(oh[:], oh[:], biasB[:])
                if h == 0:
                    outh = []
                outh.append(oh)

            # ---- output projection: y[m,f] -----------------------------
            # h-outer loop so outw streams through a small double-buffer.
            ps_y = [p_ps.tile([P, D], F32, tag="ps", name=f"ps_y{i}") for i in range(NCH)]
            for h in range(H):
                outw_t = p_outw.tile([P, D], F32, tag="outw")
                nc.sync.dma_start(outw_t[:], outwt_d[h * P : (h + 1) * P, :])
                for mc in range(NCH):
                    for half in range(2):
                        sl = slice(half * 512, (half + 1) * 512)
                        nc.tensor.matmul(
                            ps_y[mc][:, sl],
                            outh[h][:, mc * P : (mc + 1) * P],
                            outw_t[:, sl],
                            start=(h == 0),
                            stop=False,
                        )
            for mc in range(NCH):
                for half in range(2):
                    sl = slice(half * 512, (half + 1) * 512)
                    nc.tensor.matmul(
                        ps_y[mc][:, sl], ones_row[:], outb_sb[:, sl],
                        start=False, stop=True,
                    )
                yt = p_y.tile([P, D], F32, tag="y")
                nc.scalar.copy(yt[:], ps_y[mc][:])
                nc.sync.dma_start(out_d[mc * P : (mc + 1) * P, :], yt[:])

    _split_excess_waits(nc)
    return nc


_NC_CACHE = None


def _get_nc():
    global _NC_CACHE
    if _NC_CACHE is None:
        _NC_CACHE = build_nc()
    return _NC_CACHE


def _softmax_f32(z):
    z = z - z.max(axis=-1, keepdims=True)
    e = np.exp(z)
    return e / e.sum(axis=-1, keepdims=True)


def kernel(x, weight, bias, router_w, in_w, in_b, out_w, out_b):
    x = np.asarray(x, np.float32)
    weight = np.asarray(weight, np.float32)
    bias = np.asarray(bias, np.float32)
    router_w = np.asarray(router_w, np.float32)
    in_w = np.asarray(in_w, np.float32)
    out_w = np.asarray(out_w, np.float32)
    out_b = np.asarray(out_b, np.float32)

    # ---- host: router + aux loss (tiny) -------------------------------
    router_logits = x.mean(axis=1) @ router_w.T          # (B, E)
    probs = _softmax_f32(router_logits)
    top_i = np.argsort(-probs, axis=-1, kind="stable")[:, :TOP_K]  # (B, K)
    top_w = np.take_along_axis(probs, top_i, axis=-1)
    top_w = top_w / top_w.sum(axis=-1, keepdims=True)

    top1 = top_i[:, 0]
    expert_mask = np.zeros((B, E), np.float32)
    expert_mask[np.arange(B), top1] = 1.0
    aux = np.float32(E) * (probs.mean(axis=0) * expert_mask.mean(axis=0)).sum()

    # ---- host: layout prep + routing gather ---------------------------
    xt = np.ascontiguousarray(x.transpose(0, 2, 1))          # (B, D, N)
    inwt = np.ascontiguousarray(in_w.T)                      # (c, d)
    outwt = np.ascontiguousarray(out_w.T)                    # (hd, f)
    wT = np.ascontiguousarray(weight.transpose(0, 1, 3, 2))  # (E, H, n, m)

    in_maps = []
    for b in range(B):
        tw = np.empty((TOP_K, 2, N), np.float32)
        for k in range(TOP_K):
            tw[k, 0, :] = top_w[b, k]
            tw[k, 1, :] = 1.0 / top_w[b, k]
        in_maps.append(
            {
                "xt": xt[b],
                "inwt": inwt,
                "outwt": outwt,
                "outb": out_b,
                "wsel": np.ascontiguousarray(wT[top_i[b]]),
                "bsel": np.ascontiguousarray(bias[top_i[b]]),
                "tw": tw,
            }
        )

    nc = _get_nc()
    trace = bool(int(os.environ.get("MOE_KERNEL_TRACE", "0")))
    res = run_bass_kernel_spmd(nc, in_maps, list(range(B)), trace=trace)
    if trace:
        kernel.last_exec_time_ns = res.exec_time_ns
        kernel.last_profile_json = res.profile_json
        kernel.last_trace = res.instructions_and_trace

    out = np.stack([res.results[b]["out"] for b in range(B)], axis=0)
    return out, np.array(aux, dtype=np.float32)


# revision 29
# speedup vs baseline: 1.2870x; 1.2870x over previous
"""Trainium2 Bass kernel for nn_LinearMixtureOfMixers.

Strategy: data-parallel over batch B=8 across the 8 NeuronCores (one batch
element per core). The router (mean-pool -> (8,8) logits -> softmax -> top-2)
and the scalar aux loss are computed on host; routing drives a host-side
gather of each core's two (H,N,N) expert tables, which are shiped
pre-transposed so the device kernel needs no on-chip transposes anywhere:

  per core (batch b), with N=512 tokens, D=1024, H=8 heads, HD=128:
    xp[n,d]   = sum_c x[n,c] in_w[d,c]          PE  (lhsT = x.T blocks, rhs = in_w.T)
                (in_b is dropped: a per-d shift is removed exactly by the
                 token-axis layernorm that follows)
    stats     = ones-matmul partition reduction  PE  -> broadcast mu/rsqrt tiles
    xn[n,hd]  = (xp - mu) * rsqrt(var+eps)       DVE
    expw[n,m] = exp(wT[k,h])                     ACT (in-place, no max needed:
                                                      |w| <= 1/sqrt(N))
    U[d,m]    = sum_n xn[n,(h,d)] expw[n,m]      PE  (accumulate 4 n-chunks)
    rs[*,m]   = sum_n expw[n,m] / topw_k         PE  (stationary = 1/topw_k)
    outh[d,m] = U0*recip(rs0) + U1*recip(rs1) + bias_eff  DVE
    y[m,f]    = sum_hd outh[hd,m] outwT[hd,f] + out_b     PE (outh is lhsT
                 directly; out_b enters as a K=1 matmul)
"""

import os

import numpy as np

import concourse.bass as bass
import concourse.tile as tile
from concourse import mybir
from concourse.bass_utils import run_bass_kernel_spmd

F32 = mybir.dt.float32
F32R = mybir.dt.float32r
P = 128
B, N, D = 8, 512, 1024
E, H, TOP_K = 8, 8, 2
HD = D // H  # 128
NCH = N // P  # 4 n-chunks
DCH = D // P  # 8 c-chunks
EPS = 1e-5


# ---------------------------------------------------------------------------
# Workaround: this container's walrus rejects >2 sem waits on the Tile tail
# Drain. Emit one Drain per waiting proc instead (semantically identical).
def _patch_tile_drain():
    from concourse.vector_clock import ScopedClock, VectorClock

    def _drain_and_barrier_split(self, tick_clock, wait_clock):
        nc = self.nc
        vc = tick_clock.global_clock
        n = len(vc)
        for proc in range(n):
            t = vc[proc]
            if t > 0:
                single = VectorClock([t if i == proc else 0 for i in range(n)])
                d = nc.sync.drain()
                wait_clock.add_sem_waits(d.ins, ScopedClock({None: single}))
        nc.all_engine_barrier()
        assert self.sems is not None
        popped = nc._tile_sem_poison_stack.pop()
        assert popped is self._sem_poison
        nc.clear_and_free_semaphores(list(self.sems.allocated().values()))
        nc.all_engine_barrier()

    tile.TileContext._drain_and_barrier = _drain_and_barrier_split


_patch_tile_drain()


MAX_WAITS = 1  # walrus in this container caps sync waits per instruction


def _split_excess_waits(nc):
    """Hoist >MAX_WAITS sem waits onto same-engine NoOps placed just before."""
    n_split = 0
    for fn in nc.m.functions:
        for bb in fn.blocks:
            out = []
            for inst in bb.instructions:
                si = inst.sync_info
                if si is not None and si.on_wait and len(si.on_wait) > MAX_WAITS:
                    waits = list(si.on_wait)
                    extras, keep = waits[:-MAX_WAITS], waits[-MAX_WAITS:]
                    for i in range(0, len(extras), MAX_WAITS):
                        nop = mybir.InstDrain(
                            name=f"{inst.name}_wsplit{n_split}",
                            sync_info=mybir.SyncInfo(
                                on_wait=extras[i : i + MAX_WAITS], on_update=[]
                            ),
                            engine=inst.engine,
                        )
                        out.append(nop)
                        n_split += 1
                    inst.sync_info = mybir.SyncInfo(
                        on_wait=keep, on_update=si.on_update
                    )
                out.append(inst)
            bb.instructions = out
    return n_split


def build_nc():
    nc = bass.Bass()
    xt_d = nc.dram_tensor("xt", [D, N], F32, kind="ExternalInput")
    inwt_d = nc.dram_tensor("inwt", [D, D], F32, kind="ExternalInput")
    outwt_d = nc.dram_tensor("outwt", [D, D], F32, kind="ExternalInput")
    outb_d = nc.dram_tensor("outb", [D], F32, kind="ExternalInput")
    wsel_d = nc.dram_tensor("wsel", [TOP_K, H, N, N], F32, kind="ExternalInput")
    bsel_d = nc.dram_tensor("bsel", [TOP_K, H, N], F32, kind="ExternalInput")
    # tw[k, 0, :] = topw_k, tw[k, 1, :] = 1/topw_k, tw[k, 2, :] = N/topw_k
    tw_d = nc.dram_tensor("tw", [TOP_K, 3, N], F32, kind="ExternalInput")
    out_d = nc.dram_tensor("out", [N, D], F32, kind="ExternalOutput")

    with tile.TileContext(nc) as tc:
        from contextlib import ExitStack

        with ExitStack() as ctx:
            p_const = ctx.enter_context(tc.tile_pool(name="const", bufs=1))
            p_inw = ctx.enter_context(tc.tile_pool(name="inw", bufs=3))
            p_outw = ctx.enter_context(tc.tile_pool(name="outw", bufs=3))
            p_xt = ctx.enter_context(tc.tile_pool(name="xt", bufs=3))
            p_xp = ctx.enter_context(tc.tile_pool(name="xp", bufs=NCH))
            p_xn = ctx.enter_context(tc.tile_pool(name="xn", bufs=NCH))
            p_sq = ctx.enter_context(tc.tile_pool(name="sq", bufs=2))
            p_stat = ctx.enter_context(tc.tile_pool(name="stat", bufs=1))
            p_w = ctx.enter_context(tc.tile_pool(name="w", bufs=3))
            p_em1 = ctx.enter_context(tc.tile_pool(name="em1", bufs=3))
            p_outh = ctx.enter_context(tc.tile_pool(name="outh", bufs=H))
            p_small = ctx.enter_context(tc.tile_pool(name="small", bufs=2))
            p_y = ctx.enter_context(tc.tile_pool(name="y", bufs=2))
            p_ps = ctx.enter_context(tc.tile_pool(name="ps", bufs=4, space="PSUM"))
            p_dram = ctx.enter_context(tc.tile_pool(name="dram", bufs=1, space="DRAM"))

            # ---- constants & tiny inputs -------------------------------
            ones_inv = p_const.tile([P, P], F32, tag="ones_inv")
            nc.any.memset(ones_inv[:], 1.0 / N)
            eps_col = p_const.tile([P, 1], F32, tag="eps_col")
            nc.any.memset(eps_col[:], EPS)


            itw = []
            for k in range(TOP_K):
                t = p_const.tile([P, P], F32, tag=f"itw{k}")
                nc.sync.dma_start(
                    t[:].bitcast(F32R),
                    tw_d[k : k + 1, 1, 0:P].to_broadcast((P, P)).bitcast(F32R),
                )
                itw.append(t)
            twp = []
            for k in range(TOP_K):
                t = p_const.tile([H, 1], F32, tag=f"twp{k}")
                nc.sync.dma_start(t[:], tw_d[k : k + 1, 0, 0:1].to_broadcast((H, 1)))
                twp.append(t)
            c512 = []
            for k in range(TOP_K):
                t = p_const.tile([P, 1], F32, tag=f"c512_{k}")
                nc.sync.dma_start(t[:], tw_d[k : k + 1, 2, 0:1].to_broadcast((P, 1)))
                c512.append(t)

            bsk = []
            for k in range(TOP_K):
                t = p_const.tile([H, N], F32, tag=f"bsk{k}")
                nc.sync.dma_start(t[:], bsel_d[k])
                bsk.append(t)
            # bias_eff[h,m] = topw0*b0[h,m] + topw1*b1[h,m]
            beff = p_const.tile([H, N], F32, tag="beff")
            tmpb = p_const.tile([H, N], F32, tag="tmpb")
            nc.vector.tensor_scalar_mul(tmpb[:], bsk[0][:], twp[0][:])
            nc.vector.scalar_tensor_tensor(
                beff[:], bsk[1][:], twp[1][:], tmpb[:],
                mybir.AluOpType.mult, mybir.AluOpType.add,
            )
            # bounce beff to DRAM: DMA partition-broadcast needs a DRAM source
            beff_dr = p_dram.tile([H, N], F32, tag="beff_dr")
            nc.sync.dma_start(beff_dr[:], beff[:])

            # ---- input projection: xp[n,d] (4 chunks of [128, 1024]) ---
            # c-outer loop so xt/inw stream through small double-buffers.
            ps_xp = [p_ps.tile([P, D], F32, tag="ps", name=f"ps_xp{i}") for i in range(NCH)]
            for c in range(DCH):
                xt_t = p_xt.tile([P, N], F32, tag="xt")
                nc.sync.dma_start(xt_t[:], xt_d[c * P : (c + 1) * P, :])
                inw_t = p_inw.tile([P, D], F32, tag="inw")
                nc.sync.dma_start(inw_t[:], inwt_d[c * P : (c + 1) * P, :])
                for nc_i in range(NCH):
                    for half in range(2):
                        nc.tensor.matmul(
                            ps_xp[nc_i][:, half * 512 : (half + 1) * 512],
                            xt_t[:, nc_i * P : (nc_i + 1) * P],
                            inw_t[:, half * 512 : (half + 1) * 512],
                            start=(c == 0),
                            stop=(c == DCH - 1),
                        )
            xp = []
            sq = []
            for nc_i in range(NCH):
                t = p_xp.tile([P, D], F32, tag="xp")
                nc.scalar.copy(t[:], ps_xp[nc_i][:])
                xp.append(t)
                s = p_sq.tile([P, D], F32, tag="sq")
                nc.scalar.square(s[:], t[:])
                sq.append(s)

            # ---- layernorm stats over tokens (partition axis) ----------
            ps_mu = p_ps.tile([P, D], F32, tag="ps")
            ps_m2 = p_ps.tile([P, D], F32, tag="ps")
            for nc_i in range(NCH):
                for half in range(2):
                    sl = slice(half * 512, (half + 1) * 512)
                    nc.tensor.matmul(
                        ps_mu[:, sl], ones_inv[:], xp[nc_i][:, sl],
                        start=(nc_i == 0), stop=(nc_i == NCH - 1),
                    )
                    nc.tensor.matmul(
                        ps_m2[:, sl], ones_inv[:], sq[nc_i][:, sl],
                        start=(nc_i == 0), stop=(nc_i == NCH - 1),
                    )

            muB = p_stat.tile([P, D], F32, tag="muB")
            nc.scalar.copy(muB[:], ps_mu[:])
            mu2 = p_stat.tile([P, D], F32, tag="mu2")
            nc.vector.tensor_mul(mu2[:], muB[:], muB[:])
            varB = p_stat.tile([P, D], F32, tag="varB")
            nc.vector.tensor_sub(varB[:], ps_m2[:], mu2[:])
            # rsqrt(v+eps) = exp(-0.5*ln(v+eps)); ACT Ln/Exp are ~2ULP tables
            lnv = p_stat.tile([P, D], F32, tag="lnv")
            nc.scalar.activation(
                lnv[:], varB[:], mybir.ActivationFunctionType.Ln, bias=eps_col[:]
            )
            rsB = p_stat.tile([P, D], F32, tag="rsB")
            nc.scalar.activation(
                rsB[:], lnv[:], mybir.ActivationFunctionType.Exp, scale=-0.5
            )

            # xn in place of xp: (xp - mu) * rs
            xn = []
            for nc_i in range(NCH):
                nc.vector.tensor_sub(xp[nc_i][:], xp[nc_i][:], muB[:])
                t = p_xn.tile([P, D], F32, tag="xn", name=f"xn_{nc_i}")
                nc.vector.tensor_mul(t[:].bitcast(F32R), xp[nc_i][:], rsB[:])
                xn.append(t)

            # ---- experts: exp, U-einsum, rowsums, combine --------------
            for h in range(H):
                wk = []
                for k in range(TOP_K):
                    wt = p_w.tile([P, NCH, N], F32, tag="w")
                    nc.sync.dma_start(
                        wt[:],
                        wsel_d[k, h].rearrange("(c p) m -> p c m", p=P),
                    )
                    nc.scalar.activation(
                        wt[:], wt[:], mybir.ActivationFunctionType.Exp
                    )
                    # em1 = expw - 1: sum_n xn = 0 exactly (layernorm), so
                    # xn@em1 == xn@expw, and |em1|<=0.05 makes FP22
                    # truncation harmless -> float32r matmuls are safe.
                    em = p_em1.tile([P, NCH, N], F32, tag="em1", name=f"em_{h}_{k}")
                    nc.vector.tensor_scalar_sub(em[:].bitcast(F32R), wt[:], 1.0)
                    wk.append(em)

                ps_u = p_ps.tile([P, D], F32, tag="ps")  # halves: U0 | U1
                ps_r = p_ps.tile([P, D], F32, tag="ps")  # halves: rs0 | rs1
                for nc_i in range(NCH):
                    for k in range(TOP_K):
                        nc.tensor.matmul(
                            ps_u[:, k * 512 : (k + 1) * 512],
                            xn[nc_i][:, h * HD : (h + 1) * HD].bitcast(F32R),
                            wk[k][:, nc_i, :].bitcast(F32R),
                            start=(nc_i == 0),
                            stop=(nc_i == NCH - 1),
                        )
                for nc_i in range(NCH):
                    for k in range(TOP_K):
                        nc.tensor.matmul(
                            ps_r[:, k * 512 : (k + 1) * 512],
                            itw[k][:].bitcast(F32R),
                            wk[k][:, nc_i, :].bitcast(F32R),
                            start=(nc_i == 0),
                            stop=(nc_i == NCH - 1),
                        )

                # recip_k = topw_k / rowsum_k  (ps_r is already rowsum/topw):
                # 1/x = exp(-ln x) on ACT (~2ULP tables)
                rec = []
                for k in range(TOP_K):
                    lnr = p_small.tile([P, N], F32, tag="rscratch")
                    nc.scalar.activation(
                        lnr[:],
                        ps_r[:, k * 512 : (k + 1) * 512],
                        mybir.ActivationFunctionType.Ln,
                        bias=c512[k][:],
                    )
                    r = p_small.tile([P, N], F32, tag="rec")
                    nc.scalar.activation(
                        r[:], lnr[:], mybir.ActivationFunctionType.Exp, scale=-1.0
                    )
                    rec.append(r)

                biasB = p_small.tile([P, N], F32, tag="biasB")
                nc.sync.dma_start(biasB[:], beff_dr[h : h + 1, :].to_broadcast((P, N)))

                m0 = p_small.tile([P, N], F32, tag="m0")
                nc.vector.tensor_mul(m0[:], ps_u[:, 0:512], rec[0][:])
                m1 = p_small.tile([P, N], F32, tag="m1")
                nc.vector.tensor_mul(m1[:], ps_u[:, 512:1024], rec[1][:])
                oh = p_outh.tile([P, N], F32, tag="outh")
                nc.vector.tensor_add(oh[:], m0[:], m1[:])
                nc.vector.tensor_add(oh[:], oh[:], biasB[:])
                if h == 0:
                    outh = []
                outh.append(oh)

            # ---- output projection: y[m,f] -----------------------------
            # h-outer loop so outw streams through a small double-buffer.
            ps_y = [p_ps.tile([P, D], F32, tag="ps", name=f"ps_y{i}") for i in range(NCH)]
            for h in range(H):
                outw_t = p_outw.tile([P, D], F32, tag="outw")
                nc.sync.dma_start(outw_t[:], outwt_d[h * P : (h + 1) * P, :])
                for mc in range(NCH):
                    for half in range(2):
                        sl = slice(half * 512, (half + 1) * 512)
                        nc.tensor.matmul(
                            ps_y[mc][:, sl],
                            outh[h][:, mc * P : (mc + 1) * P],
                            outw_t[:, sl],
                            start=(h == 0),
                            stop=(h == H - 1),
                        )
            obB = p_const.tile([P, D], F32, tag="obB")
            nc.sync.dma_start(obB[:], outb_d[:].unsqueeze(0).to_broadcast((P, D)))
            for mc in range(NCH):
                yt = p_y.tile([P, D], F32, tag="y")
                nc.vector.tensor_add(yt[:], ps_y[mc][:], obB[:])
                nc.sync.dma_start(out_d[mc * P : (mc + 1) * P, :], yt[:])

    _split_excess_waits(nc)
    return nc


_NC_CACHE = None


def _get_nc():
    global _NC_CACHE
    if _NC_CACHE is None:
        _NC_CACHE = build_nc()
    return _NC_CACHE


def _softmax_f32(z):
    z = z - z.max(axis=-1, keepdims=True)
    e = np.exp(z)
    return e / e.sum(axis=-1, keepdims=True)


def kernel(x, weight, bias, router_w, in_w, in_b, out_w, out_b):
    x = np.asarray(x, np.float32)
    weight = np.asarray(weight, np.float32)
    bias = np.asarray(bias, np.float32)
    router_w = np.asarray(router_w, np.float32)
    in_w = np.asarray(in_w, np.float32)
    out_w = np.asarray(out_w, np.float32)
    out_b = np.asarray(out_b, np.float32)

    # ---- host: router + aux loss (tiny) -------------------------------
    router_logits = x.mean(axis=1) @ router_w.T          # (B, E)
    probs = _softmax_f32(router_logits)
    top_i = np.argsort(-probs, axis=-1, kind="stable")[:, :TOP_K]  # (B, K)
    top_w = np.take_along_axis(probs, top_i, axis=-1)
    top_w = top_w / top_w.sum(axis=-1, keepdims=True)

    top1 = top_i[:, 0]
    expert_mask = np.zeros((B, E), np.float32)
    expert_mask[np.arange(B), top1] = 1.0
    aux = np.float32(E) * (probs.mean(axis=0) * expert_mask.mean(axis=0)).sum()

    # ---- host: layout prep + routing gather ---------------------------
    xt = np.ascontiguousarray(x.transpose(0, 2, 1))          # (B, D, N)
    inwt = np.ascontiguousarray(in_w.T)                      # (c, d)
    outwt = np.ascontiguousarray(out_w.T)                    # (hd, f)
    wT = np.ascontiguousarray(weight.transpose(0, 1, 3, 2))  # (E, H, n, m)

    in_maps = []
    for b in range(B):
        tw = np.empty((TOP_K, 3, N), np.float32)
        for k in range(TOP_K):
            tw[k, 0, :] = top_w[b, k]
            tw[k, 1, :] = 1.0 / top_w[b, k]
            tw[k, 2, :] = N / top_w[b, k]
        in_maps.append(
            {
                "xt": xt[b],
                "inwt": inwt,
                "outwt": outwt,
                "outb": out_b,
                "wsel": np.ascontiguousarray(wT[top_i[b]]),
                "bsel": np.ascontiguousarray(bias[top_i[b]]),
                "tw": tw,
            }
        )

    nc = _get_nc()
    trace = bool(int(os.environ.get("MOE_KERNEL_TRACE", "0")))
    res = run_bass_kernel_spmd(nc, in_maps, list(range(B)), trace=trace)
    if trace:
        kernel.last_exec_time_ns = res.exec_time_ns
        kernel.last_profile_json = res.profile_json
        kernel.last_trace = res.instructions_and_trace

    out = np.stack([res.results[b]["out"] for b in range(B)], axis=0)
    return out, np.array(aux, dtype=np.float32)
